# revision 5
# baseline (speedup 1.0000x reference)
"""BlockWiseHistogramEncoder Trainium2 kernel (v2: packed multi-class counting).

Input  x: [16, 1, 512, 512] int32, values in [0, 64).
Output:   [16, 1024, 65] float32. Image is split into 32x32 non-overlapping
16x16 blocks (row-major block order); out[b, l, 1+v] = count(v in block l)/256,
out[b, l, 0] = 0.

Sharding: pure data parallel over batch - 2 batches per core on 8 cores.

Per-core algorithm: SBUF tiles [128 blocks, 256 elems] where cols 0:128 are
the block's first 8 rows (half 0) and cols 128:256 the last 8 (half 1).
  - VectorE: custom fused DVE op PACK3 computes, in ONE 1x pass over a
    128-elem half-block, accum += eq(x,c0) + 256*(eq(x,c1) + 256*eq(x,c2)):
    three class-counts packed base-256 into one f32 accumulator (exact:
    half-block counts <= 128, packed value < 2^24). 53 low classes ->
    17 PACK3 + 1 PACK2 ops per half-tile (211ns each, measured).
  - ScalarE: classes 53..63 via cumulative counts: ACTIVATE(Sign,
    bias=-(c-0.5), accum_out) over the full 256-col tile gives
    S'_c = #(v>=c) - #(v<c); adjacent differences recover counts.
Epilogue (VectorE + GpSimd): digits recovered with the fp32 add-magic
round trick (exact by a parity argument: ties only at digit counts that
force the quotient even), halves summed, scaled, assembled to [128, 65]
per 128-block group and DMA'd out.
"""
import sys

if "/opt/trn_rl_repo" not in sys.path:
    sys.path.insert(0, "/opt/trn_rl_repo")

import numpy as np

N_CORES = 8
B_PER_CORE = 2
H = W = 512
NC_CLS = 64
BLK = 16
HB = H // BLK          # 32 blocks per side
L = HB * HB            # 1024 blocks per batch
E = BLK * BLK          # 256 elems per block
HE = E // 2            # 128 elems per half-block
GROUPS = B_PER_CORE * L // 128   # 16 groups of 128 blocks per core

N_P3 = 17              # PACK3 ops per half-tile (classes 0..50)
N_P2 = 1               # PACK2 op per half-tile (classes 51..52)
N_DVE = 3 * N_P3 + 2 * N_P2      # 53 classes on VectorE
N_PACKS = N_P3 + N_P2            # 18 accum columns per (group, half)
N_ACT = NC_CLS - N_DVE           # 11 classes on ScalarE (53..63)
BASE = 256.0
MAGIC = 1.5 * 2.0 ** 23

_nc_cache = None
_run_cache = None


def _register_custom_ops():
    """Register PACK3/PACK2 custom DVE ops in dve_ops.OPS at runtime."""
    import concourse.dve_ops as dve_ops
    from concourse.dve_ops import DveOp, OPS, _spill_c3_to_src1
    from concourse.dve_spec import Spec, Src0, C0, C1, C2, C3, eq
    from operator import add

    have = {o.name: o for o in OPS}
    if "ANT_PACK3" in have:
        return have["ANT_PACK3"], have["ANT_PACK2"]

    def _mk(name, spec):
        op = DveOp(name, spec, subdim=False, uops_sha={})
        OPS.append(op)
        dve_ops._SUB_OPCODE_FOR_NAME[name] = (
            dve_ops._CUSTOM_DVE_ROW_BASE + len(OPS) - 1)
        assert dve_ops._SUB_OPCODE_FOR_NAME[name] < 0x20
        dve_ops.CUSTOM_DVE_SPECS[name] = spec
        real = {}
        for ver in ("v3", "v4"):
            try:
                op.compile(ver)
            except ValueError as e:
                real[ver] = str(e).split(f"{ver}: ")[1].split(" ")[0]
        op2 = DveOp(name, spec, subdim=False, uops_sha=real)
        OPS[OPS.index(op)] = op2
        dve_ops.CUSTOM_DVE_SPECS[name] = spec
        return op2

    def _pack3_ref(in0, in1, s0, s1, imm2):
        b = ((in0 == s0).astype(np.float32)
             + ((in0 == s1).astype(np.float32)
                + (in0 == in1[..., :1]).astype(np.float32) * imm2) * imm2)
        return b, b.reshape(b.shape[0], -1).sum(axis=-1, keepdims=True)

    pack3 = _mk("ANT_PACK3", Spec(
        body=_spill_c3_to_src1(
            eq(Src0, C0) + (eq(Src0, C1) + eq(Src0, C3) * C2) * C2),
        accum=add,
        reference=_pack3_ref,
    ))

    def _pack2_ref(in0, in1, s0, s1, imm2):
        b = ((in0 == s0).astype(np.float32)
             + (in0 == s1).astype(np.float32) * imm2)
        return b, b.reshape(b.shape[0], -1).sum(axis=-1, keepdims=True)

    pack2 = _mk("ANT_PACK2", Spec(
        body=eq(Src0, C0) + eq(Src0, C1) * C2,
        accum=add,
        reference=_pack2_ref,
    ))
    return pack3, pack2


def _build():
    import concourse.bacc as bacc
    import concourse.mybir as mybir
    import concourse.tile as tile

    PACK3, PACK2 = _register_custom_ops()
    f32 = mybir.dt.float32
    bf16 = mybir.dt.bfloat16

    nc = bacc.Bacc("TRN2", target_bir_lowering=False, debug=False)
    x = nc.dram_tensor("x_in", [B_PER_CORE, H, W], mybir.dt.int32,
                       kind="ExternalInput")
    y = nc.dram_tensor("y_out", [B_PER_CORE, L, NC_CLS + 1], f32,
                       kind="ExternalOutput")

    with tile.TileContext(nc) as tc:
        with tc.tile_pool(name="cst", bufs=1) as c_pool, \
             tc.tile_pool(name="io", bufs=6) as io_pool, \
             tc.tile_pool(name="wk", bufs=4) as w_pool, \
             tc.tile_pool(name="ep", bufs=2) as e_pool, \
             tc.tile_pool(name="out", bufs=4) as o_pool:
            # --- constants ---
            # ACT biases: -(c-0.5) for c = N_DVE..63
            bias = c_pool.tile([128, N_ACT], f32)
            for j in range(N_ACT):
                nc.vector.memset(bias[:, j:j + 1], -(float(N_DVE + j) - 0.5))
            # third-class values for PACK3 (digit-c class of pack p), bf16
            c3t = c_pool.tile([128, N_P3], bf16)
            for p in range(N_P3):
                nc.vector.memset(c3t[:, p:p + 1], float(3 * p + 2))
            # dummy out tiles for custom ops (values unused)
            dumps = [c_pool.tile([128, HE], f32, name=f"dump{i}")
                     for i in range(4)]
            adumps = [c_pool.tile([128, E], bf16, name=f"adump{i}")
                      for i in range(2)]

            # wide accumulators: col (g*N_PACKS + p)
            acc_h = [c_pool.tile([128, GROUPS * N_PACKS], f32, name=f"acc{h}")
                     for h in range(2)]
            # ACT cumulative sums: col (g*(N_ACT+1) + j), last col = S'_64
            SW = N_ACT + 1
            sact = c_pool.tile([128, GROUPS * SW], f32)
            for g in range(GROUPS):
                nc.gpsimd.memset(sact[:, g * SW + N_ACT: g * SW + N_ACT + 1],
                                 -float(E))

            # src views: [bh, bw, h, r, c]
            xbs = [x.ap()[b].rearrange("(bh h r) (bw c) -> bh bw h r c",
                                       h=2, r=BLK // 2, c=BLK)
                   for b in range(B_PER_CORE)]

            state = {}

            def load_stage(g):
                b, t = divmod(g, GROUPS // B_PER_CORE)
                t_in = io_pool.tile([128, E], mybir.dt.int32)
                for i in range(4):
                    dst = t_in[32 * i:32 * (i + 1), :].rearrange(
                        "bw (h r c) -> bw h r c", h=2, c=BLK)
                    nc.sync.dma_start(dst, xbs[b][4 * t + i])
                t_bf = w_pool.tile([128, E], bf16)
                nc.gpsimd.tensor_copy(t_bf[:], t_in[:])
                state[g] = t_bf

            def count_stage(g):
                t_bf = state.pop(g)
                a0 = g * N_PACKS
                for h in range(2):
                    xh = t_bf[:, HE * h:HE * (h + 1)]
                    for p in range(N_P3):
                        nc.vector._custom_dve(
                            PACK3, out=dumps[p % 4][:],
                            in0=xh, in1=c3t[:, p:p + 1],
                            s0=float(3 * p), s1=float(3 * p + 1), imm2=BASE,
                            accum_out=acc_h[h][:, a0 + p:a0 + p + 1])
                    nc.vector._custom_dve(
                        PACK2, out=dumps[3][:],
                        in0=xh, s0=float(N_DVE - 2), s1=float(N_DVE - 1),
                        imm2=BASE,
                        accum_out=acc_h[h][:, a0 + N_P3:a0 + N_P3 + 1])
                s0 = g * SW
                for j in range(N_ACT):
                    nc.scalar.activation(
                        adumps[j % 2][:], t_bf[:],
                        mybir.ActivationFunctionType.Sign,
                        bias=bias[:, j:j + 1], scale=1.0,
                        accum_out=sact[:, s0 + j:s0 + j + 1])

            # software pipeline: keep DMA/convert ahead of compute
            load_stage(0)
            load_stage(1)
            for g in range(GROUPS):
                if g + 2 < GROUPS:
                    load_stage(g + 2)
                count_stage(g)

            # ---- epilogue ----
            ts = nc.vector.tensor_scalar
            AOT = mybir.AluOpType
            WID = GROUPS * N_PACKS
            planes = []          # scaled digit planes (a, b, c) summed halves
            for d in range(3):
                planes.append(e_pool.tile([128, WID], f32, name=f"pl{d}",
                                          tag=f"pl{d}"))
            scr = [e_pool.tile([128, WID], f32, name=f"scr{i}", tag=f"s{i}")
                   for i in range(6)]
            for h in range(2):
                V = acc_h[h]
                # c digit (hi): hi3 = V*2^-16 + MAGIC
                ts(scr[0][:], V[:], 2.0 ** -16, MAGIC, AOT.mult, AOT.add)
                # hs = (hi3 - MAGIC) * 65536
                ts(scr[1][:], scr[0][:], MAGIC, 65536.0, AOT.subtract,
                   AOT.mult)
                # V2 = V - hs
                nc.vector.tensor_sub(scr[2][:], V[:], scr[1][:])
                # b digit: b3 = V2*2^-8 + MAGIC
                ts(scr[3][:], scr[2][:], 2.0 ** -8, MAGIC, AOT.mult, AOT.add)
                # bu = (b3 - MAGIC) * 256
                ts(scr[4][:], scr[3][:], MAGIC, 256.0, AOT.subtract, AOT.mult)
                # au = V2 - bu
                nc.vector.tensor_sub(scr[5][:], scr[2][:], scr[4][:])
                if h == 0:
                    # scaled digits into planes
                    ts(planes[0][:], scr[5][:], 1.0 / E, 0.0, AOT.mult,
                       AOT.add)
                    ts(planes[1][:], scr[3][:], MAGIC, 1.0 / E, AOT.subtract,
                       AOT.mult)
                    ts(planes[2][:], scr[0][:], MAGIC, 1.0 / E, AOT.subtract,
                       AOT.mult)
                else:
                    # add second half: plane += digit/E
                    t2 = e_pool.tile([128, WID], f32, tag="t2")
                    ts(t2[:], scr[5][:], 1.0 / E, 0.0, AOT.mult, AOT.add)
                    nc.vector.tensor_add(planes[0][:], planes[0][:], t2[:])
                    t3 = e_pool.tile([128, WID], f32, tag="t3")
                    ts(t3[:], scr[3][:], MAGIC, 1.0 / E, AOT.subtract,
                       AOT.mult)
                    nc.vector.tensor_add(planes[1][:], planes[1][:], t3[:])
                    t4 = e_pool.tile([128, WID], f32, tag="t4")
                    ts(t4[:], scr[0][:], MAGIC, 1.0 / E, AOT.subtract,
                       AOT.mult)
                    nc.vector.tensor_add(planes[2][:], planes[2][:], t4[:])

            # assembly + store per group
            for g in range(GROUPS):
                b, t = divmod(g, GROUPS // B_PER_CORE)
                yt = o_pool.tile([128, NC_CLS + 1], f32)
                nc.gpsimd.memset(yt[:, 0:1], 0.0)
                a0 = g * N_PACKS
                # PACK3 classes: digit d of pack p -> class 3p+d
                yv = yt[:, 1:1 + 3 * N_P3].rearrange("b (p d) -> b d p", d=3)
                for d in range(3):
                    nc.gpsimd.tensor_copy(yv[:, d], planes[d][:, a0:a0 + N_P3])
                # PACK2 classes 51, 52 (digits a, b of pack N_P3)
                nc.gpsimd.tensor_copy(
                    yt[:, 1 + N_DVE - 2:1 + N_DVE - 1],
                    planes[0][:, a0 + N_P3:a0 + N_P3 + 1])
                nc.gpsimd.tensor_copy(
                    yt[:, 1 + N_DVE - 1:1 + N_DVE],
                    planes[1][:, a0 + N_P3:a0 + N_P3 + 1])
                # ACT classes 53..63: (S'_c - S'_{c+1}) / 512
                s0 = g * SW
                dsc = o_pool.tile([128, N_ACT], f32, tag="dsc")
                nc.vector.tensor_sub(
                    dsc[:], sact[:, s0:s0 + N_ACT],
                    sact[:, s0 + 1:s0 + N_ACT + 1])
                ts(yt[:, 1 + N_DVE:1 + NC_CLS], dsc[:], 1.0 / (2 * E), 0.0,
                   AOT.mult, AOT.add)
                nc.sync.dma_start(y.ap()[b, 128 * t:128 * (t + 1)], yt[:])
    nc.compile()
    return nc


def _get_nc():
    global _nc_cache
    if _nc_cache is None:
        _nc_cache = _build()
    return _nc_cache


def _get_runner():
    """Build the sharded jitted executable once."""
    global _run_cache
    if _run_cache is not None:
        return _run_cache

    import jax
    from jax.sharding import Mesh, PartitionSpec
    from jax.experimental.shard_map import shard_map
    import concourse.mybir as mybir
    from concourse.bass2jax import (
        _bass_exec_p, install_neuronx_cc_hook, partition_id_tensor)

    nc = _get_nc()
    install_neuronx_cc_hook()

    partition_name = (nc.partition_id_tensor.name
                      if nc.partition_id_tensor else None)
    in_names, out_names, out_avals = [], [], []
    for alloc in nc.m.functions[0].allocations:
        if not isinstance(alloc, mybir.MemoryLocationSet):
            continue
        name = alloc.memorylocations[0].name
        if alloc.kind == "ExternalInput":
            if name != partition_name:
                in_names.append(name)
        elif alloc.kind == "ExternalOutput":
            out_names.append(name)
            out_avals.append(jax.core.ShapedArray(
                tuple(alloc.tensor_shape), mybir.dt.np(alloc.dtype)))
    n_params = len(in_names)
    n_outs = len(out_avals)
    all_in_names = list(in_names) + list(out_names)
    if partition_name is not None:
        all_in_names.append(partition_name)

    def _body(*args):
        operands = list(args)
        if partition_name is not None:
            operands.append(partition_id_tensor())
        outs = _bass_exec_p.bind(
            *operands,
            out_avals=tuple(out_avals),
            in_names=tuple(all_in_names),
            out_names=tuple(out_names),
            lowering_input_output_aliases=(),
            sim_require_finite=True,
            sim_require_nnan=True,
            nc=nc,
        )
        return tuple(outs)

    devices = jax.devices()[:N_CORES]
    mesh = Mesh(np.asarray(devices), ("core",))
    in_specs = (PartitionSpec("core"),) * (n_params + n_outs)
    out_specs = (PartitionSpec("core"),) * n_outs
    donate = tuple(range(n_params, n_params + n_outs))
    sharded = jax.jit(
        shard_map(_body, mesh=mesh, in_specs=in_specs, out_specs=out_specs,
                  check_rep=False),
        donate_argnums=donate, keep_unused=True)

    zero_shapes = [(N_CORES * a.shape[0], *a.shape[1:]) for a in out_avals]
    zero_dtypes = [a.dtype for a in out_avals]

    def run(concat_inputs):
        zeros = [np.zeros(s, d) for s, d in zip(zero_shapes, zero_dtypes)]
        out_arrs = sharded(*concat_inputs, *zeros)
        return {name: np.asarray(out_arrs[i]) for i, name in
                enumerate(out_names)}

    _run_cache = run
    return run


def kernel(x: np.ndarray) -> np.ndarray:
    assert x.shape == (16, 1, H, W) and x.dtype == np.int32, (x.shape, x.dtype)
    run = _get_runner()
    xs = np.ascontiguousarray(x[:, 0])          # [16, 512, 512]
    out = run([xs])["y_out"]
    return out.reshape(16, L, NC_CLS + 1).astype(np.float32, copy=False)
